# revision 9
# baseline (speedup 1.0000x reference)
"""3-layer multi-head GAT (DOMGraphTransformer) on 8 Trainium2 NeuronCores.

Strategy (graph/data parallel, dst-sharded):
  - Nodes are split into 8 contiguous shards (6250/core, padded to 6272=49*128).
  - Edges are bucketed by destination shard and, within a shard, by 128-node
    destination block; each block's edge list is padded to T_B tiles of 128.
  - Layer 1 ("mode A"): every core redundantly computes Wx1 = x0 @ W1 plus the
    per-node attention scores s1 (via the folded matrix v1 = f(W1, a1)) for
    all nodes into a local HBM table [Wx1 | s1].  Its dst shard is then
    aggregated edge-tile by edge-tile: indirect-DMA gather of Wx1[src] rows
    (one row per SBUF partition, attention scores ride along in the same
    row), a one-hot (edge x node) matrix M built on the vector engine from
    the dst column, s_dst expanded per-edge with M^T on the TensorEngine,
    per-head weights w = exp(leakyrelu(s_src+s_dst)) folded into M, and
    num/denom segment-sums accumulated in PSUM by TensorE matmuls.
  - Layers 2/3 ("mode B"): by linearity out = W.T @ (sum_e w_e x[src_e]), so
    aggregation runs per-head in input-feature space and only the 256-wide
    layer inputs travel between cores: each core's post-layer block output
    (with the next layer's folded attention scores appended) is AllGathered;
    the next layer gathers source rows from the AllGather result.  The
    per-head aggregate is projected through W after softmax normalization.
  - elu(x)+1 == relu(x) + exp(min(x,0)); the +1 cancels inside LayerNorm's
    mean subtraction, so the kernel feeds relu(x)+exp(min(x,0)) to LN.
"""
import sys
sys.path.insert(0, "/opt/trn_rl_repo")
import json as _json
import numpy as np

import concourse.bass as bass
import concourse.mybir as mybir
from concourse.tile import TileContext

# ---------------------------------------------------------------- walrus workaround
# This environment's walrus build rejects instructions carrying more than one
# semaphore wait ("Too many sync wait commands").  Split excess waits onto
# injected NoOps at BIR-JSON level.
_MAX_W = 1
_wcounter = [0]


def _split_bb(bb):
    insts = bb.get("instructions")
    if insts:
        new = []
        for ins in insts:
            si = ins.get("sync_info")
            waits = si.get("on_wait") if si else None
            if waits and len(waits) > _MAX_W:
                rest = waits[_MAX_W:]
                ins["sync_info"]["on_wait"] = waits[:_MAX_W]
                for i in range(0, len(rest), _MAX_W):
                    _wcounter[0] += 1
                    nop = {
                        "engine": ins["engine"], "ins": [],
                        "name": f"waitsplit-{_wcounter[0]}",
                        "opcode": "NoOp", "outs": [],
                        "sync_info": {"on_update": [],
                                      "on_wait": rest[i:i + _MAX_W]},
                    }
                    if "debug" in ins:
                        nop["debug"] = ins["debug"]
                    new.append(nop)
            new.append(ins)
        bb["instructions"] = new


def _walk(node):
    if isinstance(node, dict):
        if isinstance(node.get("instructions"), list):
            _split_bb(node)
        for key in ("blocks", "basic_blocks"):
            sub = node.get(key)
            if isinstance(sub, list):
                for x in sub:
                    _walk(x)


_orig_to_json_bytes = bass.Bass.to_json_bytes


def _patched_to_json_bytes(self):
    j = _json.loads(_orig_to_json_bytes(self))
    for fn in j.get("functions", []):
        _walk(fn)
    return _json.dumps(j).encode()


bass.Bass.to_json_bytes = _patched_to_json_bytes
# ----------------------------------------------------------------

F32 = mybir.dt.float32
BF16 = mybir.dt.bfloat16
I32 = mybir.dt.int32
AF = mybir.ActivationFunctionType
OP = mybir.AluOpType
AX = mybir.AxisListType

NCORES = 8
H = 4
LN_EPS = 1e-5


class Cfg:
    def __init__(self, N, dims, td=F32):
        self.N = N
        self.dims = dims              # e.g. [523, 256, 256, 512]
        self.td = td                  # table dtype for x features
        self.SH = N // NCORES
        self.NB = (self.SH + 127) // 128
        self.SHP = self.NB * 128
        self.NG = NCORES * self.SHP
        self.tdn = 1 if td == F32 else 2   # table elems per f32 (s cols)
        self.FCX = dims[1]            # x cols in every gather table
        self.SCOLS = 8 * self.tdn
        self.FC = self.FCX + self.SCOLS


def preprocess(node_feats, edge_index, Ws, as_, gs, bs, td=F32):
    N, IN_DIM = node_feats.shape
    dims = [IN_DIM] + [W.shape[1] for W in Ws]
    cfg = Cfg(N, dims, td)
    SH, NB, SHP, NG = cfg.SH, cfg.NB, cfg.SHP, cfg.NG

    src = np.asarray(edge_index[0], dtype=np.int64)
    dst = np.asarray(edge_index[1], dtype=np.int64)
    E = src.shape[0]

    core = dst // SH
    dloc = dst % SH
    blk = dloc // 128
    dcol = dloc % 128
    gb = (core * NB + blk).astype(np.int64)
    counts = np.bincount(gb, minlength=NCORES * NB)
    T_B = max(1, int(np.ceil(counts.max() / 128)))
    SLOTS = T_B * 128

    order = np.argsort(gb, kind="stable")
    starts = np.cumsum(counts) - counts
    pos_in_blk = np.arange(E) - np.repeat(starts, counts)
    remap_src = ((src // SH) * SHP + (src % SH)).astype(np.int32)

    gsrc = np.zeros((NCORES, NB, SLOTS), dtype=np.int32)
    dcol_arr = np.full((NCORES, NB, SLOTS), 999.0, dtype=np.float32)
    eg = gb[order]
    c_o, b_o = eg // NB, eg % NB
    gsrc[c_o, b_o, pos_in_blk] = remap_src[order]
    dcol_arr[c_o, b_o, pos_in_blk] = dcol[order].astype(np.float32)

    def to_pt(a):  # slot s -> tile t=s//128, partition p=s%128: [NB, 128, T_B]
        return np.ascontiguousarray(
            a.reshape(NCORES, NB, T_B, 128).transpose(0, 1, 3, 2))

    gsrc, dcol_arr = to_pt(gsrc), to_pt(dcol_arr)

    # per-core per-block padded-global row ids of the block's own 128 dst nodes
    brow = np.zeros((NCORES, NB, 128), dtype=np.int32)
    for c in range(NCORES):
        brow[c] = (c * SHP + np.arange(SHP, dtype=np.int32)).reshape(NB, 128)

    x0T = np.zeros((IN_DIM, NG), dtype=np.float32)
    xf = np.asarray(node_feats, dtype=np.float32)
    for c in range(NCORES):
        x0T[:, c * SHP:c * SHP + SH] = xf[c * SH:(c + 1) * SH].T

    vs = []
    for W, a in zip(Ws, as_):
        dout = W.shape[1]
        hd = dout // H
        Wr = W.reshape(W.shape[0], H, hd)
        vsrc = np.einsum("khd,d->kh", Wr, a[:hd])
        vdst = np.einsum("khd,d->kh", Wr, a[hd:])
        vs.append(np.concatenate([vsrc, vdst], axis=1).astype(np.float32))

    consts = {
        "x0T": x0T,
        "W1c": np.asarray(Ws[0], dtype=np.float32),
        "W2c": np.asarray(Ws[1], dtype=np.float32),
        "W3c": np.asarray(Ws[2], dtype=np.float32),
        "v1c": vs[0], "v2c": vs[1], "v3c": vs[2],
        "iota": np.broadcast_to(np.arange(128, dtype=np.float32), (128, 128)).copy(),
        "ident": np.eye(128, dtype=np.float32),
    }
    for i in range(3):
        consts[f"g{i+1}t"] = np.broadcast_to(
            np.asarray(gs[i], dtype=np.float32), (128, dims[i + 1])).copy()
        consts[f"b{i+1}t"] = np.broadcast_to(
            np.asarray(bs[i], dtype=np.float32), (128, dims[i + 1])).copy()

    in_maps = []
    for c in range(NCORES):
        m = dict(consts)
        m["gsrc"] = gsrc[c]
        m["dcol"] = dcol_arr[c]
        m["brow"] = brow[c]
        in_maps.append(m)
    return cfg, in_maps, T_B


def ceil_div(a, b):
    return (a + b - 1) // b


def build(cfg, T_B, repeat=1):
    nc = bass.Bass(trn_type="TRN2")
    td = cfg.td
    NB, SHP, NG = cfg.NB, cfg.SHP, cfg.NG
    dims = cfg.dims
    IN_DIM = dims[0]
    FCX, SCOLS, FC = cfg.FCX, cfg.SCOLS, cfg.FC

    x0T = nc.dram_tensor("x0T", [IN_DIM, NG], F32, kind="ExternalInput")
    W1c = nc.dram_tensor("W1c", [dims[0], dims[1]], F32, kind="ExternalInput")
    W2c = nc.dram_tensor("W2c", [dims[1], dims[2]], F32, kind="ExternalInput")
    W3c = nc.dram_tensor("W3c", [dims[2], dims[3]], F32, kind="ExternalInput")
    v1c = nc.dram_tensor("v1c", [dims[0], 8], F32, kind="ExternalInput")
    v2c = nc.dram_tensor("v2c", [dims[1], 8], F32, kind="ExternalInput")
    v3c = nc.dram_tensor("v3c", [dims[2], 8], F32, kind="ExternalInput")
    iota_d = nc.dram_tensor("iota", [128, 128], F32, kind="ExternalInput")
    ident_d = nc.dram_tensor("ident", [128, 128], F32, kind="ExternalInput")
    gts = [nc.dram_tensor(f"g{i+1}t", [128, dims[i + 1]], F32, kind="ExternalInput")
           for i in range(3)]
    bts = [nc.dram_tensor(f"b{i+1}t", [128, dims[i + 1]], F32, kind="ExternalInput")
           for i in range(3)]
    gsrc_d = nc.dram_tensor("gsrc", [NB, 128, T_B], I32, kind="ExternalInput")
    dcol_d = nc.dram_tensor("dcol", [NB, 128, T_B], F32, kind="ExternalInput")
    brow_d = nc.dram_tensor("brow", [NB, 128], I32, kind="ExternalInput")

    xout = nc.dram_tensor("xout", [SHP, dims[3]], F32, kind="ExternalOutput")

    wxs1_tab = nc.dram_tensor("wxs1_tab", [NG, FC], td)
    xaug_sh = [nc.dram_tensor(f"xaug{l}_sh", [SHP, FC], td) for l in (1, 2)]
    xaug_full = [nc.dram_tensor(f"xaug{l}_full", [NG, FC], td,
                                addr_space="Shared") for l in (1, 2)]

    rgroups = [list(range(NCORES))]
    kch1 = [(i * 128, min(128, IN_DIM - i * 128))
            for i in range(ceil_div(IN_DIM, 128))]

    with TileContext(nc) as tc:
        with tc.tile_pool(name="consts", bufs=1) as cpool:
            iota_sb = cpool.tile([128, 128], F32, tag="iota")
            nc.sync.dma_start(out=iota_sb[:], in_=iota_d[:, :])
            ident_sb = cpool.tile([128, 128], F32, tag="ident")
            nc.sync.dma_start(out=ident_sb[:], in_=ident_d[:, :])
            ones_sb = cpool.tile([128, 1], td, tag="ones")
            nc.gpsimd.memset(ones_sb[:], 1.0)
            for cv, ctag in ((0.0, "c0"), (LN_EPS, "ceps")):
                c_t = cpool.tile([128, 1], F32, tag=ctag)
                nc.gpsimd.memset(c_t[:], cv)
                nc.const_aps.aps[(F32, cv)] = c_t[:]

            W1_sb, v1_sb = [], []
            for i, (k0, ks) in enumerate(kch1):
                wt_ = cpool.tile([ks, dims[1]], F32, tag=f"w1_{i}")
                nc.sync.dma_start(out=wt_[:], in_=W1c[k0:k0 + ks, :])
                W1_sb.append(wt_)
                vt_ = cpool.tile([ks, 8], F32, tag=f"v1_{i}")
                nc.sync.dma_start(out=vt_[:], in_=v1c[k0:k0 + ks, :])
                v1_sb.append(vt_)
            W_sb, v_sb = {}, {}
            for l, Wd, dout in ((2, W2c, dims[2]), (3, W3c, dims[3])):
                W_sb[l] = []
                for i in range(dims[l - 1] // 128):
                    t_ = cpool.tile([128, dout], F32, tag=f"w{l}_{i}")
                    nc.sync.dma_start(out=t_[:], in_=Wd[i * 128:(i + 1) * 128, :])
                    W_sb[l].append(t_)
            for l, vd, dk in ((2, v2c, dims[1]), (3, v3c, dims[2])):
                v_sb[l] = []
                for i in range(dk // 128):
                    t_ = cpool.tile([128, 8], F32, tag=f"v{l}_{i}")
                    nc.sync.dma_start(out=t_[:], in_=vd[i * 128:(i + 1) * 128, :])
                    v_sb[l].append(t_)
            gb_sb = {}
            for i in range(3):
                g_t = cpool.tile([128, dims[i + 1]], F32, tag=f"g{i}")
                b_t = cpool.tile([128, dims[i + 1]], F32, tag=f"bb{i}")
                nc.sync.dma_start(out=g_t[:], in_=gts[i][:, :])
                nc.sync.dma_start(out=b_t[:], in_=bts[i][:, :])
                gb_sb[i + 1] = (g_t, b_t)

            # ---------- Phase A: [Wx1 | s1] table (redundant on every core)
            def phase_a(it):
              with tc.tile_pool(name=f"phA{it}", bufs=3) as pa, \
                 tc.tile_pool(name=f"phA{it}_ps", bufs=2, space="PSUM") as pap:
                for i in range(NG // 128):
                      wx_ps = pap.tile([128, dims[1]], F32, tag="wx")
                      s_ps = pap.tile([128, 8], F32, tag="s")
                      for ci, (k0, ks) in enumerate(kch1):
                          lhsT = pa.tile([128, 128], F32, tag="lhsT")
                          nc.sync.dma_start(
                              out=lhsT[:ks, :],
                              in_=x0T[k0:k0 + ks, i * 128:(i + 1) * 128])
                          nc.tensor.matmul(out=wx_ps[:], lhsT=lhsT[:ks, :],
                                           rhs=W1_sb[ci][:],
                                           start=(ci == 0), stop=(ci == len(kch1) - 1))
                          nc.tensor.matmul(out=s_ps[:], lhsT=lhsT[:ks, :],
                                           rhs=v1_sb[ci][:],
                                           start=(ci == 0), stop=(ci == len(kch1) - 1))
                      wx_sb = pa.tile([128, dims[1]], td, tag="wx_sb")
                      nc.vector.tensor_copy(out=wx_sb[:], in_=wx_ps[:])
                      s_sb = pa.tile([128, 8], F32, tag="s_sb")
                      nc.vector.tensor_copy(out=s_sb[:], in_=s_ps[:])
                      nc.sync.dma_start(out=wxs1_tab[i * 128:(i + 1) * 128, 0:FCX],
                                        in_=wx_sb[:])
                      s_dst_ap = wxs1_tab[i * 128:(i + 1) * 128, FCX:FC]
                      if td != F32:
                          nc.sync.dma_start(out=s_dst_ap, in_=s_sb[:].bitcast(td))
                      else:
                          nc.sync.dma_start(out=s_dst_ap, in_=s_sb[:])

              tc.strict_bb_all_engine_barrier()

            # ---------- edge-aggregation layers
            def edge_layer(l, it=0):
                modeA = (l == 1)
                dout = dims[l]
                F = dims[1] if modeA else dims[l - 1]
                hdA = dout // H
                tab = wxs1_tab if modeA else xaug_full[l - 2]
                aggw = dout if modeA else H * F
                gl, bl = gb_sb[l]

                with tc.tile_pool(name=f"L{l}_{it}_sb", bufs=3) as sb, \
                     tc.tile_pool(name=f"L{l}_{it}_eb", bufs=2) as eb, \
                     tc.tile_pool(name=f"L{l}_{it}_agg", bufs=2, space="PSUM") as aggp, \
                     tc.tile_pool(name=f"L{l}_{it}_den", bufs=1, space="PSUM") as denp, \
                     tc.tile_pool(name=f"L{l}_{it}_tp", bufs=1, space="PSUM") as tpp, \
                     tc.tile_pool(name=f"L{l}_{it}_sde", bufs=1, space="PSUM") as sdep, \
                     tc.tile_pool(name=f"L{l}_{it}_eps", bufs=1, space="PSUM") as epsp:
                    for b in range(NB):
                        dcol_t = eb.tile([128, T_B], F32, tag="dcol")
                        nc.sync.dma_start(out=dcol_t[:], in_=dcol_d[b, :, :])
                        xi_t = eb.tile([128, T_B], I32, tag="xi")
                        nc.sync.dma_start(out=xi_t[:], in_=gsrc_d[b, :, :])
                        bi_t = eb.tile([128, 1], I32, tag="bi")
                        nc.sync.dma_start(out=bi_t[:], in_=brow_d[b, :, None])
                        sdb = eb.tile([128, SCOLS], td, tag="sdb")
                        nc.gpsimd.indirect_dma_start(
                            out=sdb[:], out_offset=None, in_=tab[:, :],
                            in_offset=bass.IndirectOffsetOnAxis(ap=bi_t[:, :1], axis=0),
                            element_offset=FCX)
                        sdb_f = sdb[:].bitcast(F32) if td != F32 else sdb[:]

                        agg_ps = aggp.tile([128, aggw], F32, tag="agg")
                        den_ps = denp.tile([128, H], F32, tag="den")
                        # PSUM start=True zeroes a whole 2KB region: only the
                        # first/last head touching a region carries start/stop.
                        h_off = [(h * hdA if modeA else h * F) * 4 for h in range(H)]
                        h_reg = [o // 2048 for o in h_off]
                        first_h = {r: min(h for h in range(H) if h_reg[h] == r)
                                   for r in set(h_reg)}
                        last_h = {r: max(h for h in range(H) if h_reg[h] == r)
                                  for r in set(h_reg)}
                        for t in range(T_B):
                            xs_t = sb.tile([128, FC], td, tag="xs")
                            nc.gpsimd.indirect_dma_start(
                                out=xs_t[:], out_offset=None, in_=tab[:, :],
                                in_offset=bass.IndirectOffsetOnAxis(
                                    ap=xi_t[:, t:t + 1], axis=0))
                            ssrc_f = (xs_t[:, FCX:FC].bitcast(F32)
                                      if td != F32 else xs_t[:, FCX:FC])
                            m_t = sb.tile([128, 128], F32, tag="m")
                            nc.vector.tensor_tensor(
                                out=m_t[:],
                                in0=dcol_t[:, t:t + 1].to_broadcast([128, 128]),
                                in1=iota_sb[:], op=OP.is_equal)
                            mtp_ps = tpp.tile([128, 128], F32, tag="mtp")
                            nc.tensor.transpose(out=mtp_ps[:], in_=m_t[:],
                                                identity=ident_sb[:])
                            mT_sb = sb.tile([128, 128], F32, tag="mT")
                            nc.vector.tensor_copy(out=mT_sb[:], in_=mtp_ps[:])
                            sde_ps = sdep.tile([128, H], F32, tag="sde")
                            nc.tensor.matmul(out=sde_ps[:], lhsT=mT_sb[:],
                                             rhs=sdb_f[:, 4:8], start=True, stop=True)
                            ev = sb.tile([128, H], F32, tag="ev")
                            nc.vector.tensor_tensor(out=ev[:], in0=sde_ps[:],
                                                    in1=ssrc_f[:, 0:4], op=OP.add)
                            lr = sb.tile([128, H], F32, tag="lr")
                            nc.vector.tensor_scalar_mul(out=lr[:], in0=ev[:],
                                                        scalar1=0.2)
                            nc.vector.tensor_tensor(out=lr[:], in0=lr[:], in1=ev[:],
                                                    op=OP.max)
                            w_t = sb.tile([128, H], F32, tag="wt")
                            nc.scalar.activation(out=w_t[:], in_=lr[:], func=AF.Exp)
                            m4 = sb.tile([128, H * 128], td, tag="m4")
                            nc.vector.tensor_tensor(
                                out=m4[:].rearrange("p (h n) -> p h n", h=H),
                                in0=m_t[:, None, :].to_broadcast([128, H, 128]),
                                in1=w_t[:, :, None].to_broadcast([128, H, 128]),
                                op=OP.mult)
                            for h in range(H):
                                lhs = m4[:, h * 128:(h + 1) * 128]
                                if modeA:
                                    rhs = xs_t[:, h * hdA:(h + 1) * hdA]
                                    outsl = agg_ps[:, h * hdA:(h + 1) * hdA]
                                else:
                                    rhs = xs_t[:, 0:F]
                                    outsl = agg_ps[:, h * F:(h + 1) * F]
                                nc.tensor.matmul(
                                    out=outsl, lhsT=lhs, rhs=rhs,
                                    start=(t == 0 and first_h[h_reg[h]] == h),
                                    stop=(t == T_B - 1 and last_h[h_reg[h]] == h))
                                nc.tensor.matmul(
                                    out=den_ps[:, h:h + 1], lhsT=lhs,
                                    rhs=ones_sb[:, :1],
                                    start=(t == 0 and h == 0),
                                    stop=(t == T_B - 1 and h == H - 1))
                        # ---------- block epilogue
                        den_sb = sb.tile([128, H], F32, tag="den_sb")
                        nc.vector.tensor_scalar_add(out=den_sb[:], in0=den_ps[:],
                                                    scalar1=1e-8)
                        rec = sb.tile([128, H], F32, tag="rec")
                        nc.vector.reciprocal(out=rec[:], in_=den_sb[:])
                        if modeA:
                            out_pre = sb.tile([128, dout], F32, tag="opre")
                            nc.vector.tensor_tensor(
                                out=out_pre[:].rearrange("p (h d) -> p h d", h=H),
                                in0=agg_ps[:].rearrange("p (h d) -> p h d", h=H),
                                in1=rec[:, :, None].to_broadcast([128, H, hdA]),
                                op=OP.mult)
                            out_pre_ap = out_pre[:]
                        else:
                            aggn = sb.tile([128, H * F], F32, tag="aggn")
                            nc.vector.tensor_tensor(
                                out=aggn[:].rearrange("p (h d) -> p h d", h=H),
                                in0=agg_ps[:].rearrange("p (h d) -> p h d", h=H),
                                in1=rec[:, :, None].to_broadcast([128, H, F]),
                                op=OP.mult)
                            out_ps = epsp.tile([128, dout], F32, tag="eps")
                            nkc = F // 128
                            for h in range(H):
                                for kc in range(nkc):
                                    tp_ps = tpp.tile([128, 128], F32, tag="mtp")
                                    nc.tensor.transpose(
                                        out=tp_ps[:],
                                        in_=aggn[:, h * F + kc * 128:
                                                 h * F + (kc + 1) * 128],
                                        identity=ident_sb[:])
                                    tp_sb = sb.tile([128, 128], F32, tag="tp_sb")
                                    nc.vector.tensor_copy(out=tp_sb[:], in_=tp_ps[:])
                                    nc.tensor.matmul(
                                        out=out_ps[:, h * hdA:(h + 1) * hdA],
                                        lhsT=tp_sb[:],
                                        rhs=W_sb[l][kc][:, h * hdA:(h + 1) * hdA],
                                        start=(h == 0 and kc == 0),
                                        stop=(h == H - 1 and kc == nkc - 1))
                            out_pre_ap = out_ps[:]
                        # elu' = relu(x) + exp(min(x,0));  +1 cancels in LN
                        r1 = sb.tile([128, dout], F32, tag="r1")
                        nc.vector.tensor_scalar_max(out=r1[:], in0=out_pre_ap,
                                                    scalar1=0.0)
                        m1 = sb.tile([128, dout], F32, tag="m1")
                        nc.vector.tensor_scalar_min(out=m1[:], in0=out_pre_ap,
                                                    scalar1=0.0)
                        ex = sb.tile([128, dout], F32, tag="ex")
                        nc.scalar.activation(out=ex[:], in_=m1[:], func=AF.Exp)
                        y = sb.tile([128, dout], F32, tag="y")
                        nc.vector.tensor_tensor(out=y[:], in0=r1[:], in1=ex[:],
                                                op=OP.add)
                        # LayerNorm
                        mu = sb.tile([128, 1], F32, tag="mu")
                        nc.vector.tensor_reduce(out=mu[:], in_=y[:], axis=AX.X,
                                                op=OP.add)
                        nc.vector.tensor_scalar_mul(out=mu[:], in0=mu[:],
                                                    scalar1=1.0 / dout)
                        yc = sb.tile([128, dout], F32, tag="yc")
                        nc.vector.tensor_scalar(out=yc[:], in0=y[:],
                                                scalar1=mu[:, :1], scalar2=None,
                                                op0=OP.subtract)
                        sq = sb.tile([128, dout], F32, tag="sq")
                        nc.vector.tensor_tensor(out=sq[:], in0=yc[:], in1=yc[:],
                                                op=OP.mult)
                        vs_ = sb.tile([128, 1], F32, tag="vs")
                        nc.vector.tensor_reduce(out=vs_[:], in_=sq[:], axis=AX.X,
                                                op=OP.add)
                        std = sb.tile([128, 1], F32, tag="std")
                        nc.scalar.activation(out=std[:], in_=vs_[:], func=AF.Sqrt,
                                             bias=LN_EPS, scale=1.0 / dout)
                        rstd = sb.tile([128, 1], F32, tag="rstd")
                        nc.vector.reciprocal(out=rstd[:], in_=std[:])
                        yn = sb.tile([128, dout], F32, tag="yn")
                        nc.vector.tensor_scalar(out=yn[:], in0=yc[:],
                                                scalar1=rstd[:, :1], scalar2=None,
                                                op0=OP.mult)
                        z = sb.tile([128, dout], F32, tag="z")
                        nc.vector.tensor_tensor(out=z[:], in0=yn[:], in1=gl[:],
                                                op=OP.mult)
                        nc.vector.tensor_tensor(out=z[:], in0=z[:], in1=bl[:],
                                                op=OP.add)
                        # ---------- outputs
                        if l == 3:
                            nc.sync.dma_start(out=xout[b * 128:(b + 1) * 128, :],
                                              in_=z[:])
                        else:
                            if td != F32:
                                z_td = sb.tile([128, dout], td, tag="z_td")
                                nc.vector.tensor_copy(out=z_td[:], in_=z[:])
                                z_out = z_td
                            else:
                                z_out = z
                            nc.sync.dma_start(
                                out=xaug_sh[l - 1][b * 128:(b + 1) * 128, 0:FCX],
                                in_=z_out[:])
                            s_ps = epsp.tile([128, 8], F32, tag="eps")
                            for kc in range(dout // 128):
                                tp2 = tpp.tile([128, 128], F32, tag="mtp")
                                nc.tensor.transpose(
                                    out=tp2[:],
                                    in_=z[:, kc * 128:(kc + 1) * 128],
                                    identity=ident_sb[:])
                                tp2_sb = sb.tile([128, 128], F32, tag="tp_sb")
                                nc.vector.tensor_copy(out=tp2_sb[:], in_=tp2[:])
                                nc.tensor.matmul(out=s_ps[:], lhsT=tp2_sb[:],
                                                 rhs=v_sb[l + 1][kc][:],
                                                 start=(kc == 0),
                                                 stop=(kc == dout // 128 - 1))
                            s_sb2 = sb.tile([128, 8], F32, tag="s_sb2")
                            nc.vector.tensor_copy(out=s_sb2[:], in_=s_ps[:])
                            s_out = xaug_sh[l - 1][b * 128:(b + 1) * 128, FCX:FC]
                            if td != F32:
                                nc.sync.dma_start(out=s_out, in_=s_sb2[:].bitcast(td))
                            else:
                                nc.sync.dma_start(out=s_out, in_=s_sb2[:])
                if l != 3:
                    nc.gpsimd.collective_compute(
                        "AllGather", OP.bypass, replica_groups=rgroups,
                        ins=[xaug_sh[l - 1][:, :]], outs=[xaug_full[l - 1][:, :]])
                    tc.strict_bb_all_engine_barrier()

            for it in range(repeat):
                phase_a(it)
                edge_layer(1, it)
                edge_layer(2, it)
                edge_layer(3, it)

    return nc


def kernel(**inputs):
    node_feats = np.asarray(inputs["node_feats"], dtype=np.float32)
    edge_index = np.asarray(inputs["edge_index"])
    Ws = [np.asarray(inputs[f"W{i}"], dtype=np.float32) for i in (1, 2, 3)]
    as_ = [np.asarray(inputs[f"a{i}"], dtype=np.float32) for i in (1, 2, 3)]
    gs = [np.asarray(inputs[f"g{i}"], dtype=np.float32) for i in (1, 2, 3)]
    bs = [np.asarray(inputs[f"b{i}"], dtype=np.float32) for i in (1, 2, 3)]

    cfg, in_maps, T_B = preprocess(node_feats, edge_index, Ws, as_, gs, bs, td=F32)
    nc = build(cfg, T_B)

    from concourse.bass_utils import run_bass_kernel_spmd
    res = run_bass_kernel_spmd(nc, in_maps, list(range(NCORES)))
    SH = cfg.SH
    out = np.concatenate([res.results[c]["xout"][:SH] for c in range(NCORES)], axis=0)
    return out.astype(np.float32)
